# revision 1
# baseline (speedup 1.0000x reference)
"""LIF neuron (no reset) Trainium2 kernel.

h_t = 0.5*h_{t-1} + 0.5*x_t ; spike_t = (h_t >= 1.0), x: [T=32, B=64, N=32768] f32.

Sharding: pure data-parallel over batch dim (dim 1) across 8 NeuronCores;
each core scans its [32, 8, 32768] shard over time. Per timestep the
262144-element slab is viewed as [128 partitions, 2048]; 4 timesteps are
moved per DMA (4 MiB).

Shipped kernel (build_program_v2): scaled recurrence S_t = S_{t-1} + 2^t x_t
(one DVE scalar_tensor_tensor per step) with spike_t = (S_t >= 2^{t+1})
written as uint8 {0,1} and widened to f32 on the host. Scaling the
reference's h_t = fl(fl(0.5h)+fl(0.5x)) = fl(0.5(h+x)) chain by the exact
power of two 2^t commutes with round-to-nearest, so the kernel is
bit-exact vs the jax reference (verified: 0/67108864 mismatches).

Measured on the 8 axon trn2 cores (reps-slope timing): the per-core HBM
read path caps at ~250 GB/s (vs ~376 GB/s for writes), so the 32 MiB/core
input load dominates; uint8 spikes cut the store stream 4x and hide it
behind the loads. f32 spike stores are fully additive with loads (+90 us).
"""

import numpy as np

import concourse.bass as bass
import concourse.mybir as mybir
import concourse.tile as tile
from concourse import bacc
from concourse.bass_utils import run_bass_kernel_spmd

T, B, N = 32, 64, 32768
NCORES = 8
B_SH = B // NCORES            # 8 batch rows per core
E = B_SH * N                  # 262144 elements per timestep per core
P = 128                       # SBUF partitions
STEPS_PER_CHUNK = 4

_prog_cache: dict = {}


def build_program(reps: int = 1, t_steps: int = T, e: int = E,
                  steps_per_chunk: int = STEPS_PER_CHUNK,
                  variant: str = "full", isge_engine: str = "vector",
                  xp_bufs: int | None = None):
    """Per-core Bass program: x[t_steps, e] f32 -> y[t_steps, e] f32 (spikes).

    reps>1 repeats the whole scan (h re-zeroed each rep) for wall-clock
    HW timing: t(reps=K)-t(reps=1) ~= (K-1)*kernel_time, cancelling RPC
    and host-transfer overhead. Output stays correct (last rep wins).
    """
    f = e // P
    nchunks = t_steps // steps_per_chunk
    assert e % P == 0 and t_steps % steps_per_chunk == 0

    nc = bacc.Bacc()
    x = nc.declare_dram_parameter("x", [t_steps, e], mybir.dt.float32, isOutput=False)
    y = nc.declare_dram_parameter("y", [t_steps, e], mybir.dt.float32, isOutput=True)

    if xp_bufs is None:
        # chunk tile is spc*f*4 B/partition; keep pool under ~170 KiB
        xp_bufs = max(2, min(4, (170 * 1024) // (steps_per_chunk * (e // P) * 4)))
    with tile.TileContext(nc) as tc:
        with (
            tc.tile_pool(name="xp", bufs=xp_bufs) as xp,
            tc.tile_pool(name="tp", bufs=2) as tp,
            tc.tile_pool(name="hp", bufs=1) as hp,
        ):
            h = hp.tile([P, f], mybir.dt.float32, name="h")

            def body(_i=None):
                nc.vector.memset(h[:], 0.0)
                if variant == "computeonly":
                    # same op schedule, no DMA: one resident x chunk + a
                    # separate spike tile (mimics the real dep structure)
                    isge = getattr(nc, isge_engine)
                    xr = xp.tile([P, steps_per_chunk, f], mybir.dt.float32,
                                 name="xr")
                    nc.vector.memset(xr[:], 0.25)
                    for c in range(nchunks):
                        sc = xp.tile([P, steps_per_chunk, f],
                                     mybir.dt.float32, name="sc", tag="sc",
                                     bufs=2)
                        for dt in range(steps_per_chunk):
                            tmp = tp.tile([P, f], mybir.dt.float32,
                                          name="tmp", tag="tmp")
                            nc.vector.tensor_add(tmp[:], h[:], xr[:, dt, :])
                            nc.vector.tensor_scalar_mul(h[:], tmp[:], 0.5)
                            isge.tensor_scalar(
                                sc[:, dt, :], tmp[:], 2.0, None,
                                mybir.AluOpType.is_ge)
                    return
                if variant == "storeonly":
                    xs = xp.tile([P, steps_per_chunk, f], mybir.dt.float32,
                                 name="xs")
                    nc.vector.memset(xs[:], 1.0)
                    for c in range(nchunks):
                        t0 = c * steps_per_chunk
                        nc.scalar.dma_start(
                            y[t0:t0 + steps_per_chunk, :].rearrange(
                                "t (p f) -> p t f", p=P),
                            xs[:],
                        )
                    return
                for c in range(nchunks):
                    t0 = c * steps_per_chunk
                    xc = xp.tile([P, steps_per_chunk, f], mybir.dt.float32,
                                 name="xc", tag="xc")
                    # loads ride the SP HWDGE ring, stores the ACT ring, so
                    # the two directions overlap instead of serializing on
                    # one descriptor FIFO
                    nc.sync.dma_start(
                        xc[:],
                        x[t0:t0 + steps_per_chunk, :].rearrange(
                            "t (p f) -> p t f", p=P),
                    )
                    if variant == "full":
                        isge = getattr(nc, isge_engine)
                        for dt in range(steps_per_chunk):
                            tmp = tp.tile([P, f], mybir.dt.float32,
                                          name="tmp", tag="tmp")
                            nc.vector.tensor_add(tmp[:], h[:], xc[:, dt, :])
                            nc.vector.tensor_scalar_mul(h[:], tmp[:], 0.5)
                            # spikes overwrite the consumed x slice in place
                            isge.tensor_scalar(
                                xc[:, dt, :], tmp[:], 2.0, None,
                                mybir.AluOpType.is_ge)
                    elif variant == "fused":
                        # Scaled recurrence: S_t = S_{t-1} + 2^t x_t, spike
                        # = (S_t >= 2^{t+1}).  Scaling both addends of the
                        # reference's h_t = fl(h+x)/2 by 2^t commutes with
                        # round-to-nearest, so this is bit-exact with the
                        # jax reference while needing one DVE op per step
                        # for the recurrence instead of two.  2^32 ~ 4.3e9
                        # stays far from fp32 overflow.
                        for dt in range(steps_per_chunk):
                            t_abs = t0 + dt
                            nc.vector.scalar_tensor_tensor(
                                h[:], xc[:, dt, :], float(2.0 ** t_abs),
                                h[:], mybir.AluOpType.mult,
                                mybir.AluOpType.add)
                            nc.vector.tensor_scalar(
                                xc[:, dt, :], h[:],
                                float(2.0 ** (t_abs + 1)), None,
                                mybir.AluOpType.is_ge)
                    if variant == "loadonly":
                        # token store keeps xc alive without write traffic
                        nc.scalar.dma_start(
                            y[t0, :P * 2].rearrange("(p f) -> p f", p=P),
                            xc[:, 0, :2])
                    else:
                        nc.scalar.dma_start(
                            y[t0:t0 + steps_per_chunk, :].rearrange(
                                "t (p f) -> p t f", p=P),
                            xc[:],
                        )

            if reps == 1:
                body()
            else:
                with tc.For_i(0, reps, 1) as i:
                    body(i)
    nc.compile()
    return nc


def build_program_v2(reps: int = 1, t_steps: int = T, e: int = E,
                     steps_per_chunk: int = STEPS_PER_CHUNK, bufs: int = 3,
                     out_u8: bool = True, isge_engine: str = "vector",
                     load_rings: tuple = ("sync",), store_ring: str = "scalar",
                     prologue: bool = False):
    """v2: scaled single-op recurrence + uint8 spike output.

    S_t = S_{t-1} + 2^t x_t  (one scalar_tensor_tensor per step);
    spike_t = (S_t >= 2^{t+1}) written as uint8 {0,1}.  Scaling the
    reference's h_t = fl(h+x)/2 by 2^t commutes with round-to-nearest
    (power-of-two scales are exact), so this is bit-exact with the jax
    reference.  2^32 ~ 4.3e9 stays far from fp32 overflow.

    Spikes land in their own uint8 tile (not in-place over x), so the
    store stream depends only on the is_ge ops and x buffers recycle as
    soon as the last scalar_tensor_tensor of the chunk has read them.
    """
    f = e // P
    spc = steps_per_chunk
    assert e % P == 0 and t_steps % spc == 0
    if prologue and spc == 4 and t_steps >= 8:
        # split the first chunk so compute starts after a 0.5 MiB load
        # instead of 4 MiB (cuts the pipeline-fill latency)
        sched = [1, 1, 2] + [spc] * ((t_steps - spc) // spc)
    else:
        sched = [spc] * (t_steps // spc)

    nc = bacc.Bacc()
    x = nc.declare_dram_parameter("x", [t_steps, e], mybir.dt.float32,
                                  isOutput=False)
    ydt = mybir.dt.uint8 if out_u8 else mybir.dt.float32
    y = nc.declare_dram_parameter("y", [t_steps, e], ydt, isOutput=True)

    with tile.TileContext(nc) as tc:
        with (
            tc.tile_pool(name="xp", bufs=bufs) as xp,
            tc.tile_pool(name="sp", bufs=bufs) as sp,
            tc.tile_pool(name="hp", bufs=1) as hp,
        ):
            h = hp.tile([P, f], mybir.dt.float32, name="h")
            if isge_engine == "alt":
                isge_of = lambda t: nc.gpsimd if t % 2 else nc.vector
            else:
                isge_of = lambda t, e=getattr(nc, isge_engine): e

            def body(_i=None):
                nc.vector.memset(h[:], 0.0)
                t0 = 0
                for c, spc_c in enumerate(sched):
                    xc = xp.tile([P, spc_c, f], mybir.dt.float32, name="xc",
                                 tag="xc")
                    sc = sp.tile([P, spc_c, f], ydt, name="sc", tag="sc")
                    getattr(nc, load_rings[c % len(load_rings)]).dma_start(
                        xc[:],
                        x[t0:t0 + spc_c, :].rearrange("t (p f) -> p t f", p=P),
                    )
                    for dt in range(spc_c):
                        t_abs = t0 + dt
                        nc.vector.scalar_tensor_tensor(
                            h[:], xc[:, dt, :], float(2.0 ** t_abs), h[:],
                            mybir.AluOpType.mult, mybir.AluOpType.add)
                        isge_of(t_abs).tensor_scalar(
                            sc[:, dt, :], h[:], float(2.0 ** (t_abs + 1)),
                            None, mybir.AluOpType.is_ge)
                    getattr(nc, store_ring).dma_start(
                        y[t0:t0 + spc_c, :].rearrange("t (p f) -> p t f", p=P),
                        sc[:],
                    )
                    t0 += spc_c

            if reps == 1:
                body()
            else:
                with tc.For_i(0, reps, 1) as i:
                    body(i)
    nc.compile()
    return nc


def run_sharded(x: np.ndarray, nc) -> np.ndarray:
    """Shard [T,B,N] over batch across 8 cores, run, gather."""
    in_maps = [
        {"x": np.ascontiguousarray(x[:, i * B_SH:(i + 1) * B_SH, :]).reshape(T, E)}
        for i in range(NCORES)
    ]
    res = run_bass_kernel_spmd(nc, in_maps, list(range(NCORES)))
    out = np.empty((T, B, N), dtype=np.float32)
    for i, r in enumerate(res.results):
        out[:, i * B_SH:(i + 1) * B_SH, :] = r["y"].reshape(T, B_SH, N)
    return out


def build_main_program(reps: int = 1):
    """The shipped configuration (single place to keep test.py in sync).

    spc=1 won the in-process sweeps (vs spc2/spc4/spc8): 1 MiB load
    chunks with deep buffering pipeline the capped HBM read path best.
    bufs=12 beat bufs=10 by 2.2 us/rep (~123.5 us total).
    """
    return build_program_v2(reps=reps, steps_per_chunk=1, bufs=12)


def kernel(x_seq: np.ndarray) -> np.ndarray:
    x = np.asarray(x_seq, dtype=np.float32)
    assert x.shape == (T, B, N), x.shape
    if "main" not in _prog_cache:
        _prog_cache["main"] = build_main_program()
    return run_sharded(x, _prog_cache["main"])



# revision 2
# speedup vs baseline: 6.1718x; 6.1718x over previous
"""LIF neuron (no reset) Trainium2 kernel, v3 (bit-packed spike output).

h_t = 0.5*h_{t-1} + 0.5*x_t ; spike_t = (h_t >= 1.0), x: [T=32, B=64, N=32768] f32.

Sharding: pure data-parallel over batch dim (dim 1) across 8 NeuronCores;
each core scans its [32, 8, 32768] shard over time, with each timestep's
262144-element slab viewed as [128 partitions, 2048].

The kernel is HBM-read-bound (32 MiB/core of fp32 input at ~320 GB/s/core
under 8-core contention ~= 103 us), so v3 minimizes everything else:

  * DVE runs ONLY the scaled recurrence S_t = S_{t-1} + 2^t x_t (one
    scalar_tensor_tensor per step, double-buffered S so the chain never
    waits on other engines' reads).  Scaling the reference's
    h_t = fl(fl(0.5h)+fl(0.5x)) chain by the exact power of two 2^t
    commutes with round-to-nearest, so S is bit-exact vs the reference
    and spike_t = (S_t >= 2^{t+1}).
  * ACT computes sg_t = Sign(S_t - 2^{t+1}) in {-1,0,+1} (bf16).
  * PE accumulates word += (2^k I) @ sg_t into PSUM (fp32, exact: all
    addends are distinct powers of two <= 2^15).
  * After each 16-step half: mask = (word + 65535)/2 is the exact 16-bit
    spike mask (ACT Copy with scale=0.5/bias=32767.5, uint16 out), DMA'd
    out.  Store traffic is 1 MiB/core instead of 8 MiB (u8) / 32 MiB (f32).

Host side unpacks the masks to f32 spikes.  Exact-tie elements
(S_t == 2^{t+1} bit-for-bit, where Sign is 0, probability ~4e-8/element)
decode with <=2 flipped bits; measured 2 mismatches per 8.4M elements --
far inside the 2e-2 rel-err gate.

Measured (reps-slope, 8 cores concurrent, steady-state): ~106-110 us vs
~127-138 us for the v2 uint8 baseline in the same process; pure-load
floor is ~103 us.
"""

import numpy as np

import concourse.bass as bass
import concourse.mybir as mybir
import concourse.tile as tile
from concourse import bacc
from concourse.bass_utils import run_bass_kernel_spmd

T, B, N = 32, 64, 32768
NCORES = 8
B_SH = B // NCORES            # 8 batch rows per core
E = B_SH * N                  # 262144 elements per timestep per core
P = 128                       # SBUF partitions
F = E // P                    # 2048 free-dim columns
FB = 512                      # fp32 columns per PSUM bank

_prog_cache: dict = {}


def build_program_v3(reps: int = 1, bufs: int = 12, sg_bufs: int = 4):
    """Per-core Bass program: x[T, E] f32 -> w[2, E] u16 spike bitmasks.

    w[half, e] bit k = spike at t = 16*half + k.

    reps>1 repeats the whole scan (S re-zeroed each rep) inside a
    hardware For_i loop for wall-clock HW timing: t(reps=K)-t(reps=J)
    ~= (K-J)*kernel_time, cancelling RPC and host-transfer overhead.
    """
    nc = bacc.Bacc()
    x = nc.declare_dram_parameter("x", [T, E], mybir.dt.float32,
                                  isOutput=False)
    ident = nc.declare_dram_parameter("ident", [16 * P, P],
                                      mybir.dt.bfloat16, isOutput=False)
    w = nc.declare_dram_parameter("w", [2, E], mybir.dt.uint16,
                                  isOutput=True)

    with tile.TileContext(nc) as tc:
        with (
            tc.tile_pool(name="xp", bufs=bufs) as xp,
            tc.tile_pool(name="sgp", bufs=sg_bufs) as sgp,
            tc.tile_pool(name="hp", bufs=1) as hp,
            tc.tile_pool(name="wp", bufs=2) as wp,
            tc.tile_pool(name="pp", bufs=1, space="PSUM") as pp,
        ):
            S2 = hp.tile([P, 2, F], mybir.dt.float32, name="S2")
            idt = hp.tile([P, 16, P], mybir.dt.bfloat16, name="idt")
            wlo = pp.tile([P, F], mybir.dt.float32, name="wlo")
            whi = pp.tile([P, F], mybir.dt.float32, name="whi")
            # 16 scaled identities 2^k * I, loaded once per program
            nc.sync.dma_start(idt[:], ident.rearrange("(k p) q -> p k q", p=P))
            # per-step ACT bias constants -(2^(t+1)) as [P,1] columns
            biases = hp.tile([P, T], mybir.dt.float32, name="biases")
            for t in range(T):
                nc.gpsimd.memset(biases[:, t:t + 1], float(-(2.0 ** (t + 1))))

            def body(_i=None):
                # double-buffered S: step t writes S2[:,t%2] reading
                # S2[:,(t+1)%2], so ACT's read of step t never blocks the
                # DVE write of step t+1 (no cross-engine WAR ping-pong)
                nc.vector.memset(S2[:, 1, :], 0.0)
                for t in range(T):
                    s_prev = S2[:, (t + 1) % 2, :]
                    s_cur = S2[:, t % 2, :]
                    xc = xp.tile([P, 1, F], mybir.dt.float32, name="xc",
                                 tag="xc")
                    nc.sync.dma_start(
                        xc[:],
                        x[t:t + 1, :].rearrange("t (p f) -> p t f", p=P))
                    nc.vector.scalar_tensor_tensor(
                        s_cur, xc[:, 0, :], float(2.0 ** t), s_prev,
                        mybir.AluOpType.mult, mybir.AluOpType.add)
                    sg = sgp.tile([P, F], mybir.dt.bfloat16, name="sg",
                                  tag="sg")
                    nc.scalar.activation(
                        sg[:], s_cur, mybir.ActivationFunctionType.Sign,
                        bias=biases[:, t:t + 1], scale=1.0)
                    word = wlo if t < 16 else whi
                    k = t % 16
                    for j in range(4):
                        nc.tensor.matmul(
                            word[:, j * FB:(j + 1) * FB],
                            idt[:, k, :],
                            sg[:, j * FB:(j + 1) * FB],
                            start=(k == 0), stop=(k == 15))
                    if k == 15:
                        wu = wp.tile([P, F], mybir.dt.uint16, name="wu",
                                     tag="wu")
                        # mask = (word + 65535)/2, an exact integer in
                        # [0, 65535]; drain on ACT so the DVE chain and
                        # the PE never wait on it
                        nc.scalar.activation(
                            wu[:], word[:],
                            mybir.ActivationFunctionType.Copy,
                            bias=32767.5, scale=0.5)
                        nc.scalar.dma_start(
                            w[t // 16, :].rearrange("(p f) -> p f", p=P),
                            wu[:])

            if reps == 1:
                body()
            else:
                with tc.For_i(0, reps, 1) as i:
                    body(i)
    nc.compile()
    return nc


def make_ident() -> np.ndarray:
    """[16*128, 128] bf16: block k is 2^k * I_128."""
    from ml_dtypes import bfloat16
    ident = np.zeros((16 * P, P), dtype=np.float32)
    eye = np.eye(P, dtype=np.float32)
    for k in range(16):
        ident[k * P:(k + 1) * P] = eye * (2.0 ** k)
    return ident.astype(bfloat16)


def decode_masks(w: np.ndarray) -> np.ndarray:
    """[2, E] u16 bitmasks -> [T, E] f32 spikes."""
    mask = w.astype(np.uint32)                       # [2, E]
    ks = np.arange(16, dtype=np.uint32)
    bits = (mask[:, None, :] >> ks[None, :, None]) & 1   # [2, 16, E]
    return bits.reshape(T, -1).astype(np.float32)    # t = 16*half + k


def run_sharded(x: np.ndarray, nc) -> np.ndarray:
    """Shard [T,B,N] over batch across 8 cores, run, gather + unpack."""
    ident = make_ident()
    in_maps = [
        {
            "x": np.ascontiguousarray(
                x[:, i * B_SH:(i + 1) * B_SH, :]).reshape(T, E),
            "ident": ident,
        }
        for i in range(NCORES)
    ]
    res = run_bass_kernel_spmd(nc, in_maps, list(range(NCORES)))
    out = np.empty((T, B, N), dtype=np.float32)
    for i, r in enumerate(res.results):
        out[:, i * B_SH:(i + 1) * B_SH, :] = decode_masks(
            r["w"]).reshape(T, B_SH, N)
    return out


def build_main_program(reps: int = 1):
    """The shipped configuration (single place to keep test.py in sync).

    bufs=12 beat 14/16 (deeper prefetch congests the contended HBM read
    path); single sync-ring loads beat sync+scalar alternation; the
    PSUM drain on ACT keeps DVE free.
    """
    return build_program_v3(reps=reps, bufs=12, sg_bufs=4)


def kernel(x_seq: np.ndarray) -> np.ndarray:
    x = np.asarray(x_seq, dtype=np.float32)
    assert x.shape == (T, B, N), x.shape
    if "main" not in _prog_cache:
        _prog_cache["main"] = build_main_program()
    return run_sharded(x, _prog_cache["main"])


# revision 4
# speedup vs baseline: 7.2953x; 1.1821x over previous
"""LIF neuron (no reset) Trainium2 kernel, v3 (bit-packed spike output).

h_t = 0.5*h_{t-1} + 0.5*x_t ; spike_t = (h_t >= 1.0), x: [T=32, B=64, N=32768] f32.

Sharding: pure data-parallel over batch dim (dim 1) across 8 NeuronCores;
each core scans its [32, 8, 32768] shard over time, with each timestep's
262144-element slab viewed as [128 partitions, 2048].

The kernel is HBM-read-bound (32 MiB/core of fp32 input at ~320 GB/s/core
under 8-core contention ~= 103 us), so v3 minimizes everything else:

  * DVE runs ONLY the scaled recurrence S_t = S_{t-1} + 2^t x_t (one
    scalar_tensor_tensor per step, double-buffered S so the chain never
    waits on other engines' reads).  Scaling the reference's
    h_t = fl(fl(0.5h)+fl(0.5x)) chain by the exact power of two 2^t
    commutes with round-to-nearest, so S is bit-exact vs the reference
    and spike_t = (S_t >= 2^{t+1}).
  * ACT computes sg_t = Sign(S_t - 2^{t+1}) in {-1,0,+1} (bf16).
  * PE accumulates word += (2^k I) @ sg_t into PSUM (fp32, exact: all
    addends are distinct powers of two <= 2^15).
  * After each 16-step half: mask = (word + 65535)/2 is the exact 16-bit
    spike mask (ACT Copy with scale=0.5/bias=32767.5, uint16 out), DMA'd
    out.  Store traffic is 1 MiB/core instead of 8 MiB (u8) / 32 MiB (f32).

Host side unpacks the masks to f32 spikes.  Exact-tie elements
(S_t == 2^{t+1} bit-for-bit, where Sign is 0, probability ~4e-8/element)
decode with <=2 flipped bits; measured 2 mismatches per 8.4M elements --
far inside the 2e-2 rel-err gate.

Measured (reps-slope, 8 cores concurrent, steady-state): ~106-110 us vs
~127-138 us for the v2 uint8 baseline in the same process; pure-load
floor is ~103 us.
"""

import numpy as np

import concourse.bass as bass
import concourse.mybir as mybir
import concourse.tile as tile
from concourse import bacc
from concourse.bass_utils import run_bass_kernel_spmd

T, B, N = 32, 64, 32768
NCORES = 8
B_SH = B // NCORES            # 8 batch rows per core
E = B_SH * N                  # 262144 elements per timestep per core
P = 128                       # SBUF partitions
F = E // P                    # 2048 free-dim columns
FB = 512                      # fp32 columns per PSUM bank

_prog_cache: dict = {}


def build_program_v3(reps: int = 1, bufs: int = 12, sg_bufs: int = 4):
    """Per-core Bass program: x[T, E] f32 -> w[2, E] u16 spike bitmasks.

    w[half, e] bit k = spike at t = 16*half + k.

    reps>1 repeats the whole scan (S re-zeroed each rep) inside a
    hardware For_i loop for wall-clock HW timing: t(reps=K)-t(reps=J)
    ~= (K-J)*kernel_time, cancelling RPC and host-transfer overhead.
    """
    nc = bacc.Bacc()
    x = nc.declare_dram_parameter("x", [T, E], mybir.dt.float32,
                                  isOutput=False)
    ident = nc.declare_dram_parameter("ident", [16 * P, P],
                                      mybir.dt.bfloat16, isOutput=False)
    w = nc.declare_dram_parameter("w", [2, E], mybir.dt.uint16,
                                  isOutput=True)

    with tile.TileContext(nc) as tc:
        with (
            tc.tile_pool(name="xp", bufs=bufs) as xp,
            tc.tile_pool(name="sgp", bufs=sg_bufs) as sgp,
            tc.tile_pool(name="hp", bufs=1) as hp,
            tc.tile_pool(name="wp", bufs=2) as wp,
            tc.tile_pool(name="pp", bufs=1, space="PSUM") as pp,
        ):
            S2 = hp.tile([P, 2, F], mybir.dt.float32, name="S2")
            idt = hp.tile([P, 16, P], mybir.dt.bfloat16, name="idt")
            wlo = pp.tile([P, F], mybir.dt.float32, name="wlo")
            whi = pp.tile([P, F], mybir.dt.float32, name="whi")
            # 16 scaled identities 2^k * I, loaded once per program
            nc.sync.dma_start(idt[:], ident.rearrange("(k p) q -> p k q", p=P))
            # per-step ACT bias constants -(2^(t+1)) as [P,1] columns
            biases = hp.tile([P, T], mybir.dt.float32, name="biases")
            for t in range(T):
                nc.gpsimd.memset(biases[:, t:t + 1], float(-(2.0 ** (t + 1))))

            def body(_i=None):
                # double-buffered S: step t writes S2[:,t%2] reading
                # S2[:,(t+1)%2], so ACT's read of step t never blocks the
                # DVE write of step t+1 (no cross-engine WAR ping-pong)
                nc.vector.memset(S2[:, 1, :], 0.0)
                for t in range(T):
                    s_prev = S2[:, (t + 1) % 2, :]
                    s_cur = S2[:, t % 2, :]
                    xc = xp.tile([P, 1, F], mybir.dt.float32, name="xc",
                                 tag="xc")
                    nc.sync.dma_start(
                        xc[:],
                        x[t:t + 1, :].rearrange("t (p f) -> p t f", p=P))
                    nc.vector.scalar_tensor_tensor(
                        s_cur, xc[:, 0, :], float(2.0 ** t), s_prev,
                        mybir.AluOpType.mult, mybir.AluOpType.add)
                    sg = sgp.tile([P, F], mybir.dt.bfloat16, name="sg",
                                  tag="sg")
                    nc.scalar.activation(
                        sg[:], s_cur, mybir.ActivationFunctionType.Sign,
                        bias=biases[:, t:t + 1], scale=1.0)
                    word = wlo if t < 16 else whi
                    k = t % 16
                    for j in range(4):
                        nc.tensor.matmul(
                            word[:, j * FB:(j + 1) * FB],
                            idt[:, k, :],
                            sg[:, j * FB:(j + 1) * FB],
                            start=(k == 0), stop=(k == 15))
                    if k == 15:
                        wu = wp.tile([P, F], mybir.dt.uint16, name="wu",
                                     tag="wu")
                        # mask = (word + 65535)/2, an exact integer in
                        # [0, 65535]; drain on ACT so the DVE chain and
                        # the PE never wait on it
                        nc.scalar.activation(
                            wu[:], word[:],
                            mybir.ActivationFunctionType.Copy,
                            bias=32767.5, scale=0.5)
                        nc.scalar.dma_start(
                            w[t // 16, :].rearrange("(p f) -> p f", p=P),
                            wu[:])

            if reps == 1:
                body()
            else:
                with tc.For_i(0, reps, 1) as i:
                    body(i)
    nc.compile()
    return nc


def make_ident() -> np.ndarray:
    """[16*128, 128] bf16: block k is 2^k * I_128."""
    from ml_dtypes import bfloat16
    ident = np.zeros((16 * P, P), dtype=np.float32)
    eye = np.eye(P, dtype=np.float32)
    for k in range(16):
        ident[k * P:(k + 1) * P] = eye * (2.0 ** k)
    return ident.astype(bfloat16)


def decode_masks(w: np.ndarray) -> np.ndarray:
    """[2, E] u16 bitmasks -> [T, E] f32 spikes."""
    mask = w.astype(np.uint32)                       # [2, E]
    ks = np.arange(16, dtype=np.uint32)
    bits = (mask[:, None, :] >> ks[None, :, None]) & 1   # [2, 16, E]
    return bits.reshape(T, -1).astype(np.float32)    # t = 16*half + k


def run_sharded(x: np.ndarray, nc) -> np.ndarray:
    """Shard [T,B,N] over batch across 8 cores, run, gather + unpack."""
    ident = make_ident()
    in_maps = [
        {
            "x": np.ascontiguousarray(
                x[:, i * B_SH:(i + 1) * B_SH, :]).reshape(T, E),
            "ident": ident,
        }
        for i in range(NCORES)
    ]
    res = run_bass_kernel_spmd(nc, in_maps, list(range(NCORES)))
    out = np.empty((T, B, N), dtype=np.float32)
    for i, r in enumerate(res.results):
        out[:, i * B_SH:(i + 1) * B_SH, :] = decode_masks(
            r["w"]).reshape(T, B_SH, N)
    return out


def build_main_program(reps: int = 1):
    """The shipped configuration (single place to keep test.py in sync).

    bufs=12 beat 14/16 (deeper prefetch congests the contended HBM read
    path); single sync-ring loads beat sync+scalar alternation; the
    PSUM drain on ACT keeps DVE free.
    """
    return build_program_v3(reps=reps, bufs=12, sg_bufs=4)


def kernel(x_seq: np.ndarray) -> np.ndarray:
    x = np.asarray(x_seq, dtype=np.float32)
    assert x.shape == (T, B, N), x.shape
    if "main" not in _prog_cache:
        _prog_cache["main"] = build_main_program()
    return run_sharded(x, _prog_cache["main"])
